# revision 22
# baseline (speedup 1.0000x reference)
import os
import sys
import tempfile

sys.path.insert(0, "/opt/trn_rl_repo")

# persistent XLA compilation cache: the per-call jit of run_bass_kernel_spmd
# re-lowers the same NEFF-wrapped executable every call; cache it on disk
_JAX_CACHE = os.path.join(tempfile.gettempdir(), "jax_comp_cache")
os.environ.setdefault("JAX_COMPILATION_CACHE_DIR", _JAX_CACHE)
os.environ.setdefault("JAX_PERSISTENT_CACHE_MIN_COMPILE_TIME_SECS", "0")

import numpy as np
import ml_dtypes

import jax

try:
    jax.config.update("jax_compilation_cache_dir", _JAX_CACHE)
    jax.config.update("jax_persistent_cache_min_compile_time_secs", 0.0)
except Exception:
    pass

import concourse.bass as bass
import concourse.mybir as mybir
import concourse.tile as tile
from concourse import bacc
from concourse.bass_utils import run_bass_kernel_spmd

# Problem constants (hardcoded per contract)
N_CORES = 8
B = 32
B_LOC = B // N_CORES  # 4 batches per core
S = 484
E = 1024
H = 1024  # q proj dim = 16 heads * 64
KV = 256  # kv proj dim = 4 groups * 64
G = 4
HKV = 4
NH = 16
D = 64
MD = 484  # MAX_DIST
TW = 2 * MD - 1  # 967 table rows
DW = 968  # bias window width per head
PW = 1096  # padded reversed rel-table row width
F32 = mybir.dt.float32
F16 = mybir.dt.float16
U8 = mybir.dt.uint8
I8 = mybir.dt.int8

# s tiling: 484 = 128*3 + 100
ST = [(0, 128), (128, 128), (256, 128), (384, 100)]
NE = E // 128  # 8 contraction tiles

# wb column layout: [Wq | Wk | Wv | Wo]
WQ0, WK0, WV0, WO0, WB_W = 0, 1024, 1280, 1536, 2560

# wp: flat fp16 blob = per-core weight slice | pd table
NW = 128 * WB_W
NP = NH * PW
WP_N = NW + NP
# xsc: per-(b, t, e-tile, s-tile) f32 scales, [128, B_LOC*3*NE*4]
XSC_W = B_LOC * 3 * NE * 4
# flat u8 output blob layout (per core): out | per-row f32 scales (as bytes)
NXO = B_LOC * S * E
OB_N = NXO + B_LOC * S * 4


def build_nc():
    nc = bacc.Bacc("TRN2", target_bir_lowering=False, debug=False, num_devices=N_CORES)

    xb = nc.dram_tensor("xb", [B_LOC, 3, E, S], I8, kind="ExternalInput")
    xsc = nc.dram_tensor("xsc", [128, XSC_W], F32, kind="ExternalInput")
    wp = nc.dram_tensor("wp", [WP_N], F16, kind="ExternalInput")
    ob = nc.dram_tensor("ob", [OB_N], U8, kind="ExternalOutput")

    from contextlib import ExitStack

    with tile.TileContext(nc) as tc:
        with ExitStack() as ctx:
            wbp = ctx.enter_context(tc.tile_pool(name="wbp", bufs=1))
            bdp = ctx.enter_context(tc.tile_pool(name="bdp", bufs=1))
            xep = ctx.enter_context(tc.tile_pool(name="xe", bufs=4))
            xip = ctx.enter_context(tc.tile_pool(name="xi", bufs=4))
            xrp = ctx.enter_context(tc.tile_pool(name="xr", bufs=4))
            xtp = ctx.enter_context(tc.tile_pool(name="xt", bufs=48))
            qtp = ctx.enter_context(tc.tile_pool(name="qt", bufs=8))
            kdp = ctx.enter_context(tc.tile_pool(name="kd", bufs=4))
            vhp = ctx.enter_context(tc.tile_pool(name="vh", bufs=4))
            pfp = ctx.enter_context(tc.tile_pool(name="pf", bufs=6))
            pbp = ctx.enter_context(tc.tile_pool(name="pb", bufs=3))
            otp = ctx.enter_context(tc.tile_pool(name="ot", bufs=8))
            osp = ctx.enter_context(tc.tile_pool(name="os", bufs=2))
            oup = ctx.enter_context(tc.tile_pool(name="ou", bufs=2))
            lvp = ctx.enter_context(tc.tile_pool(name="lv", bufs=2))
            lbp = ctx.enter_context(tc.tile_pool(name="lb", bufs=2))
            rmp = ctx.enter_context(tc.tile_pool(name="rm", bufs=4))
            rsp = ctx.enter_context(tc.tile_pool(name="rs", bufs=4))
            psA = ctx.enter_context(tc.tile_pool(name="psA", bufs=6, space="PSUM"))
            psB = ctx.enter_context(tc.tile_pool(name="psB", bufs=2, space="PSUM"))

            # --- resident weights: AllGather the 8 per-core row slices, then load ---
            dramp = ctx.enter_context(tc.tile_pool(name="dram", bufs=1, space="DRAM"))
            wg_in = dramp.tile([128, WB_W], F16, tag="wgi")
            wg_out = dramp.tile([E, WB_W], F16, tag="wgo")
            nc.gpsimd.dma_start(
                wg_in[:], bass.AP(wp, 0, [[WB_W, 128], [1, WB_W]])
            )
            nc.gpsimd.collective_compute(
                "AllGather",
                mybir.AluOpType.bypass,
                replica_groups=[list(range(N_CORES))],
                ins=[wg_in.opt()],
                outs=[wg_out.opt()],
            )
            wb_sb = []
            for e in range(NE):
                t = wbp.tile([128, WB_W], F16, tag="wb", name="wb_t", bufs=8)
                nc.sync.dma_start(out=t[:], in_=wg_out[e * 128:(e + 1) * 128, :])
                wb_sb.append(t)
            xs_sb = wbp.tile([128, XSC_W], F32, tag="xs")
            nc.sync.dma_start(out=xs_sb[:], in_=xsc[:, :])

            def wq_ap(e, h0, h1):
                return wb_sb[e][:, WQ0 + h0:WQ0 + h1]

            def wk_ap(e, m0, m1):
                return wb_sb[e][:, WK0 + m0:WK0 + m1]

            def wv_ap(e):
                return wb_sb[e][:, WV0:WV0 + KV]

            def wo_ap(e, n0, n1):
                return wb_sb[e][:, WO0 + n0:WO0 + n1]

            # --- bias windows: D[h, i, c] = rel[i + 966 - c] = pd[h, 127 - i + c]
            # DMA loads overlapping diagonals E0[j, c] = pd[h, j + c] (all strides +1),
            # then a PE matmul against a reversal permutation flips the partition order.
            rv = bdp.tile([128, 128], F16, tag="rv")
            nc.gpsimd.memset(rv[:], 0.0)
            nc.gpsimd.affine_select(
                out=rv[:],
                in_=rv[:],
                compare_op=mybir.AluOpType.not_equal,
                fill=1.0,
                base=-127,
                pattern=[[1, 128]],
                channel_multiplier=1,
            )
            bd_sb = bdp.tile([128, NH * DW], F16, tag="bd")
            for h in range(NH):
                e0 = xep.tile([128, DW], F16, tag="e0", name="e0_t")
                nc.sync.dma_start(
                    out=e0[:],
                    in_=bass.AP(wp, NW + h * PW, [[1, 128], [1, DW]]),
                )
                for c0, c1 in ((0, 512), (512, DW)):
                    psr = psB.tile([128, 512], F32, tag="psB", name="psB_t")
                    nc.tensor.matmul(
                        psr[:, 0:c1 - c0], rv[:], e0[:, c0:c1], start=True, stop=True
                    )
                    nc.scalar.copy(
                        bd_sb[:, h * DW + c0:h * DW + c1], psr[:, 0:c1 - c0]
                    )

            for b in range(B_LOC):
                # ------------- load + dequantize x tiles for this batch -------------
                # int8 -> fp16 raw (gpsimd cast) -> scale per s-tile (DVE)
                xt = [[None] * NE for _ in range(3)]
                for t in range(3):
                    for e in range(NE):
                        xi8 = xip.tile([128, S], I8, tag="xi", name="xi_t")
                        nc.sync.dma_start(
                            out=xi8[:], in_=xb[b, t, e * 128:(e + 1) * 128, :]
                        )
                        xraw = xrp.tile([128, S], F16, tag="xr", name="xr_t")
                        nc.gpsimd.tensor_copy(xraw[:], xi8[:])
                        xd = xtp.tile([128, S], F16, tag="xt", name="xt_t")
                        col = ((b * 3 + t) * NE + e) * 4
                        for si, (s0, sl) in enumerate(ST):
                            nc.vector.tensor_scalar(
                                out=xd[:, s0:s0 + sl], in0=xraw[:, s0:s0 + sl],
                                scalar1=xs_sb[:, col + si:col + si + 1], scalar2=None,
                                op0=mybir.AluOpType.mult,
                            )
                        xt[t][e] = xd

                # ---------------- K^T and V-hat ----------------
                kps = [psA.tile([128, S], F32, tag="psA", name="psA_t") for _ in range(2)]
                vps = [psA.tile([128, KV], F32, tag="psA", name="psA_v") for _ in range(4)]
                for e in range(NE):
                    xke = xt[1][e]
                    xve = xt[2][e]
                    st = e == 0
                    sp = e == NE - 1
                    for m in range(2):
                        nc.tensor.matmul(
                            kps[m][:],
                            wk_ap(e, m * 128, (m + 1) * 128),
                            xke[:],
                            start=st,
                            stop=sp,
                        )
                    for si, (s0, sl) in enumerate(ST):
                        nc.tensor.matmul(
                            vps[si][0:sl, :],
                            xve[:, s0:s0 + sl],
                            wv_ap(e),
                            start=st,
                            stop=sp,
                        )
                # evac K^T into per-group duplicated tiles (group at rows 0-63 AND 64-127)
                kd_sb = [kdp.tile([128, S], F16, tag="kd", name="kd_t") for _ in range(G)]
                for g in range(G):
                    src = kps[g // 2][(g % 2) * 64:(g % 2) * 64 + 64, :]
                    nc.scalar.copy(kd_sb[g][0:64, :], src)
                    nc.scalar.copy(kd_sb[g][64:128, :], src)
                # evac V into [128, G, 65] tiles with ones column
                vh_sb = []
                for si, (s0, sl) in enumerate(ST):
                    t = vhp.tile([128, G, 65], F16, tag="vh", name="vh_t")
                    for g in range(G):
                        nc.vector.memset(t[:, g, 64:65], 1.0)
                    nc.scalar.copy(
                        t[0:sl, :, 0:64],
                        vps[si][0:sl, :].rearrange("p (g d) -> p g d", g=G),
                    )
                    vh_sb.append(t)

                # ---------------- Q^T (2 rounds of 4 h-tiles) ----------------
                qt_sb = [qtp.tile([128, S], F16, tag="qt", name="qt_t") for _ in range(NE)]
                for rnd in range(2):
                    qps = [psA.tile([128, S], F32, tag="psA", name="psA_t") for _ in range(4)]
                    for e in range(NE):
                        for hi in range(4):
                            ht = rnd * 4 + hi
                            nc.tensor.matmul(
                                qps[hi][:],
                                wq_ap(e, ht * 128, (ht + 1) * 128),
                                xt[0][e][:],
                                start=(e == 0),
                                stop=(e == NE - 1),
                            )
                    for hi in range(4):
                        nc.vector.tensor_copy(qt_sb[rnd * 4 + hi][:], qps[hi][:])

                # ---------------- attention per head ----------------
                ot_sb = [otp.tile([128, S], F16, tag="ot", name="ot_t") for _ in range(NE)]
                for hh in range(NH):
                    g = hh // HKV
                    base = (hh % 2) * 64
                    q_ap = qt_sb[hh // 2][base:base + 64, :]
                    p_bf = pbp.tile([128, 4, S], F16, tag="pb", name="pb_t")
                    for si, (s0, sl) in enumerate(ST):
                        sps = psA.tile([128, S], F32, tag="psA", name="psA_t")
                        nc.tensor.matmul(
                            sps[0:sl, :],
                            kd_sb[g][base:base + 64, s0:s0 + sl],
                            q_ap,
                            start=True,
                            stop=True,
                        )
                        # logits = s * 0.125 + bias (f32), then exp -> fp16 on ACT
                        p_f = pfp.tile([128, S], F32, tag="pf", name="pf_t")
                        nc.vector.scalar_tensor_tensor(
                            p_f[0:sl, :],
                            sps[0:sl, :],
                            0.125,
                            bd_sb[0:sl, hh * DW + (MD - 1 - s0):hh * DW + (MD - 1 - s0) + S],
                            op0=mybir.AluOpType.mult,
                            op1=mybir.AluOpType.add,
                        )
                        nc.scalar.activation(
                            p_bf[0:sl, si, :],
                            p_f[0:sl, :],
                            mybir.ActivationFunctionType.Exp,
                        )
                    ops = psB.tile([128, 512], F32, tag="psB", name="psB_t")
                    for si, (s0, sl) in enumerate(ST):
                        nc.tensor.matmul(
                            ops[0:65, 0:S],
                            vh_sb[si][0:sl, g, :],
                            p_bf[0:sl, si, :],
                            start=(si == 0),
                            stop=(si == 3),
                        )
                    linv = lvp.tile([1, S], F32, tag="lv", name="lv_t")
                    nc.vector.reciprocal(linv[:], ops[64:65, 0:S])
                    lbc = lbp.tile([64, S], F32, tag="lb", name="lb_t")
                    nc.gpsimd.partition_broadcast(lbc[:], linv[:])
                    nc.vector.tensor_mul(
                        ot_sb[hh // 2][base:base + 64, :],
                        ops[0:64, 0:S],
                        lbc[:],
                    )

                # ---------------- output projection + uint8 quantization ----------------
                for si, (s0, sl) in enumerate(ST):
                    accs = []
                    for n in range(2):
                        acc = psA.tile([128, 512], F32, tag="psA", name="psA_q")
                        for dt in range(NE):
                            nc.tensor.matmul(
                                acc[0:sl, :],
                                ot_sb[dt][:, s0:s0 + sl],
                                wo_ap(dt, n * 512, (n + 1) * 512),
                                start=(dt == 0),
                                stop=(dt == NE - 1),
                            )
                        accs.append(acc)
                    # per-row absmax over both halves -> scale = absmax/127
                    rm = rmp.tile([128, 4], F32, tag="rm", name="rm_t")
                    for n in range(2):
                        nc.vector.tensor_reduce(
                            rm[0:sl, n:n + 1],
                            accs[n][0:sl, :],
                            axis=mybir.AxisListType.X,
                            op=mybir.AluOpType.max,
                            apply_absolute_value=True,
                        )
                    nc.vector.tensor_tensor(
                        rm[0:sl, 2:3], rm[0:sl, 0:1], rm[0:sl, 1:2],
                        op=mybir.AluOpType.max,
                    )
                    rsc = rsp.tile([128, 1], F32, tag="rs", name="rs_t")
                    nc.vector.tensor_scalar(
                        out=rsc[0:sl, :], in0=rm[0:sl, 2:3],
                        scalar1=1e-20, scalar2=1.0 / 127.0,
                        op0=mybir.AluOpType.max, op1=mybir.AluOpType.mult,
                    )
                    nc.sync.dma_start(
                        out=bass.AP(ob, NXO + (b * S + s0) * 4, [[4, sl], [1, 4]]),
                        in_=rsc[0:sl, :].bitcast(U8),
                    )
                    nc.vector.reciprocal(rm[0:sl, 3:4], rsc[0:sl, :])
                    for n in range(2):
                        stg = osp.tile([128, 512], F16, tag="os", name="os_t")
                        nc.vector.tensor_scalar(
                            out=stg[0:sl, :], in0=accs[n][0:sl, :],
                            scalar1=rm[0:sl, 3:4], scalar2=128.0,
                            op0=mybir.AluOpType.mult, op1=mybir.AluOpType.add,
                        )
                        stu = oup.tile([128, 512], U8, tag="ou", name="ou_t")
                        nc.gpsimd.tensor_copy(stu[0:sl, :], stg[0:sl, :])
                        nc.sync.dma_start(
                            out=bass.AP(
                                ob, (b * S + s0) * E + n * 512, [[E, sl], [1, 512]]
                            ),
                            in_=stu[0:sl, :],
                        )

    nc.compile()
    return nc


_NC = None


def _get_nc():
    global _NC
    if _NC is None:
        _NC = build_nc()
    return _NC


def _host_prep(query, key, value, Wq, Wk, Wv, Wo, rel_table):
    # int8 per-(batch, tensor, channel, s-tile) symmetric quantization of x^T
    X = np.empty((B, 3, E, S), dtype=np.int8)
    SC = np.empty((B, 3, E, 4), dtype=np.float32)
    for t, a in enumerate((query, key, value)):
        at = a.transpose(0, 2, 1)  # [B, E, S] view
        for si, (s0, sl) in enumerate(ST):
            blk = at[:, :, s0:s0 + sl]
            amax = np.maximum(np.abs(blk).max(axis=2), 1e-20)  # [B, E]
            sc = (amax / 127.0).astype(np.float32)
            SC[:, t, :, si] = sc
            q = np.rint(blk / sc[:, :, None])
            np.clip(q, -127, 127, out=q)
            X[:, t, :, s0:s0 + sl] = q
    # xsc layout: [p, ((b*3+t)*NE+e)*4+si] per core (b local)
    xs_all = np.ascontiguousarray(
        SC.reshape(B, 3, NE, 128, 4).transpose(3, 0, 1, 2, 4).reshape(128, B * 3 * NE * 4)
    )

    wbm = np.empty((E, WB_W), dtype=np.float16)
    wbm[:, WQ0:WQ0 + H] = Wq
    wbm[:, WK0:WK0 + KV] = Wk
    wbm[:, WV0:WV0 + KV] = Wv
    wbm[:, WO0:WO0 + E] = Wo
    # pd[h, m] = rel_table[1093 - m, h] for m in [127, 1093], else 0
    pdv = np.zeros((NH, PW), dtype=np.float16)
    pdv[:, 127:127 + TW] = rel_table[::-1, :].T

    ncols = B_LOC * 3 * NE * 4
    in_maps = []
    for c in range(N_CORES):
        sl = slice(c * B_LOC, (c + 1) * B_LOC)
        wp_c = np.concatenate(
            [wbm[c * 128:(c + 1) * 128].reshape(-1), pdv.reshape(-1)]
        )
        in_maps.append(
            {
                "xb": X[sl],
                "xsc": np.ascontiguousarray(xs_all[:, c * ncols:(c + 1) * ncols]),
                "wp": wp_c,
            }
        )
    return in_maps


def _run(inputs, trace=False):
    nc = _get_nc()
    in_maps = _host_prep(**inputs)
    res = run_bass_kernel_spmd(
        nc, in_maps, list(range(N_CORES)), trace=trace
    )
    outs = []
    for r in res.results:
        blob = r["ob"]
        u = blob[:NXO].reshape(B_LOC, S, E).astype(np.float32)
        sc = blob[NXO:].view(np.float32).reshape(B_LOC, S)
        u -= 128.0
        u *= sc[:, :, None]
        outs.append(u)
    outp = np.concatenate(outs, axis=0)
    return outp, res


def kernel(query, key, value, Wq, Wk, Wv, Wo, rel_table):
    outp, _ = _run(
        dict(
            query=np.asarray(query),
            key=np.asarray(key),
            value=np.asarray(value),
            Wq=np.asarray(Wq),
            Wk=np.asarray(Wk),
            Wv=np.asarray(Wv),
            Wo=np.asarray(Wo),
            rel_table=np.asarray(rel_table),
        )
    )
    return outp


# revision 29
# speedup vs baseline: 1.0279x; 1.0279x over previous
import os
import sys
import tempfile

sys.path.insert(0, "/opt/trn_rl_repo")

# persistent XLA compilation cache: the per-call jit of run_bass_kernel_spmd
# re-lowers the same NEFF-wrapped executable every call; cache it on disk
_JAX_CACHE = os.path.join(tempfile.gettempdir(), "jax_comp_cache")
os.environ.setdefault("JAX_COMPILATION_CACHE_DIR", _JAX_CACHE)
os.environ.setdefault("JAX_PERSISTENT_CACHE_MIN_COMPILE_TIME_SECS", "0")

import numpy as np
import ml_dtypes

import jax

try:
    jax.config.update("jax_compilation_cache_dir", _JAX_CACHE)
    jax.config.update("jax_persistent_cache_min_compile_time_secs", 0.0)
except Exception:
    pass

import concourse.bass as bass
import concourse.mybir as mybir
import concourse.tile as tile
from concourse import bacc
from concourse.bass_utils import run_bass_kernel_spmd

# Problem constants (hardcoded per contract)
N_CORES = 8
B = 32
B_LOC = B // N_CORES  # 4 batches per core
S = 484
E = 1024
H = 1024  # q proj dim = 16 heads * 64
KV = 256  # kv proj dim = 4 groups * 64
G = 4
HKV = 4
NH = 16
D = 64
MD = 484  # MAX_DIST
TW = 2 * MD - 1  # 967 table rows
DW = 968  # bias window width per head
PW = 1096  # padded reversed rel-table row width
F32 = mybir.dt.float32
F16 = mybir.dt.float16
U8 = mybir.dt.uint8
I8 = mybir.dt.int8

# s tiling: 484 = 128*3 + 100
ST = [(0, 128), (128, 128), (256, 128), (384, 100)]
NE = E // 128  # 8 contraction tiles

# wb column layout: [Wq | Wk | Wv | Wo]
WQ0, WK0, WV0, WO0, WB_W = 0, 1024, 1280, 1536, 2560

# wp: flat fp16 blob = per-core weight slice | pd table
NW = 128 * WB_W
NP = NH * PW
WP_N = NW + NP
# xsc: per-(b, t, e-tile, s-tile) f32 scales, [128, B_LOC*3*NE*4]
XSC_W = B_LOC * 3 * NE * 4
# flat u8 output blob layout (per core): out | per-row f32 scales (as bytes)
NXO = B_LOC * S * E
OB_N = NXO + B_LOC * S * 4


def build_nc():
    nc = bacc.Bacc("TRN2", target_bir_lowering=False, debug=False, num_devices=N_CORES)

    xb = nc.dram_tensor("xb", [B_LOC, 3, E, S], I8, kind="ExternalInput")
    xsc = nc.dram_tensor("xsc", [128, XSC_W], F16, kind="ExternalInput")
    wp = nc.dram_tensor("wp", [WP_N], F16, kind="ExternalInput")
    ob = nc.dram_tensor("ob", [OB_N], U8, kind="ExternalOutput")

    from contextlib import ExitStack

    with tile.TileContext(nc) as tc:
        with ExitStack() as ctx:
            wbp = ctx.enter_context(tc.tile_pool(name="wbp", bufs=1))
            bdp = ctx.enter_context(tc.tile_pool(name="bdp", bufs=1))
            xep = ctx.enter_context(tc.tile_pool(name="xe", bufs=4))
            xip = ctx.enter_context(tc.tile_pool(name="xi", bufs=6))
            xrp = ctx.enter_context(tc.tile_pool(name="xr", bufs=6))
            xtp = ctx.enter_context(tc.tile_pool(name="xt", bufs=48))
            qtp = ctx.enter_context(tc.tile_pool(name="qt", bufs=8))
            kdp = ctx.enter_context(tc.tile_pool(name="kd", bufs=4))
            vhp = ctx.enter_context(tc.tile_pool(name="vh", bufs=4))
            pfp = ctx.enter_context(tc.tile_pool(name="pf", bufs=6))
            pbp = ctx.enter_context(tc.tile_pool(name="pb", bufs=3))
            otp = ctx.enter_context(tc.tile_pool(name="ot", bufs=8))
            osp = ctx.enter_context(tc.tile_pool(name="os", bufs=2))
            oup = ctx.enter_context(tc.tile_pool(name="ou", bufs=2))
            lvp = ctx.enter_context(tc.tile_pool(name="lv", bufs=2))
            lbp = ctx.enter_context(tc.tile_pool(name="lb", bufs=2))
            rmp = ctx.enter_context(tc.tile_pool(name="rm", bufs=4))
            rsp = ctx.enter_context(tc.tile_pool(name="rs", bufs=4))
            psA = ctx.enter_context(tc.tile_pool(name="psA", bufs=6, space="PSUM"))
            psB = ctx.enter_context(tc.tile_pool(name="psB", bufs=2, space="PSUM"))

            # --- resident weights: AllGather the 8 per-core row slices, then load ---
            dramp = ctx.enter_context(tc.tile_pool(name="dram", bufs=1, space="DRAM"))
            wg_in = dramp.tile([128, WB_W], F16, tag="wgi")
            wg_out = dramp.tile([E, WB_W], F16, tag="wgo")
            nc.gpsimd.dma_start(
                wg_in[:], bass.AP(wp, 0, [[WB_W, 128], [1, WB_W]])
            )
            nc.gpsimd.collective_compute(
                "AllGather",
                mybir.AluOpType.bypass,
                replica_groups=[list(range(N_CORES))],
                ins=[wg_in.opt()],
                outs=[wg_out.opt()],
            )
            wb_sb = []
            for e in range(NE):
                t = wbp.tile([128, WB_W], F16, tag="wb", name="wb_t", bufs=8)
                nc.sync.dma_start(out=t[:], in_=wg_out[e * 128:(e + 1) * 128, :])
                wb_sb.append(t)
            xs16 = wbp.tile([128, XSC_W], F16, tag="xs16")
            nc.sync.dma_start(out=xs16[:], in_=xsc[:, :])
            xs_sb = wbp.tile([128, XSC_W], F32, tag="xs")
            nc.scalar.copy(xs_sb[:], xs16[:])

            def wq_ap(e, h0, h1):
                return wb_sb[e][:, WQ0 + h0:WQ0 + h1]

            def wk_ap(e, m0, m1):
                return wb_sb[e][:, WK0 + m0:WK0 + m1]

            def wv_ap(e):
                return wb_sb[e][:, WV0:WV0 + KV]

            def wo_ap(e, n0, n1):
                return wb_sb[e][:, WO0 + n0:WO0 + n1]

            # --- bias windows: D[h, i, c] = rel[i + 966 - c] = pd[h, 127 - i + c]
            # DMA loads overlapping diagonals E0[j, c] = pd[h, j + c] (all strides +1),
            # then a PE matmul against a reversal permutation flips the partition order.
            rv = bdp.tile([128, 128], F16, tag="rv")
            nc.gpsimd.memset(rv[:], 0.0)
            nc.gpsimd.affine_select(
                out=rv[:],
                in_=rv[:],
                compare_op=mybir.AluOpType.not_equal,
                fill=1.0,
                base=-127,
                pattern=[[1, 128]],
                channel_multiplier=1,
            )
            bd_sb = bdp.tile([128, NH * DW], F16, tag="bd")
            for h in range(NH):
                e0 = xep.tile([128, DW], F16, tag="e0", name="e0_t")
                nc.sync.dma_start(
                    out=e0[:],
                    in_=bass.AP(wp, NW + h * PW, [[1, 128], [1, DW]]),
                )
                for c0, c1 in ((0, 512), (512, DW)):
                    psr = psB.tile([128, 512], F32, tag="psB", name="psB_t")
                    nc.tensor.matmul(
                        psr[:, 0:c1 - c0], rv[:], e0[:, c0:c1], start=True, stop=True
                    )
                    nc.scalar.copy(
                        bd_sb[:, h * DW + c0:h * DW + c1], psr[:, 0:c1 - c0]
                    )

            for b in range(B_LOC):
                # ------------- load + dequantize x tiles for this batch -------------
                # int8 -> fp16 raw (gpsimd cast) -> scale per s-tile (DVE)
                xt = [[None] * NE for _ in range(3)]
                for t in range(3):
                    for e in range(NE):
                        xi8 = xip.tile([128, S], I8, tag="xi", name="xi_t")
                        nc.sync.dma_start(
                            out=xi8[:], in_=xb[b, t, e * 128:(e + 1) * 128, :]
                        )
                        xraw = xrp.tile([128, S], F16, tag="xr", name="xr_t")
                        nc.gpsimd.tensor_copy(xraw[:], xi8[:])
                        xd = xtp.tile([128, S], F16, tag="xt", name="xt_t")
                        col = ((b * 3 + t) * NE + e) * 4
                        for si, (s0, sl) in enumerate(ST):
                            nc.vector.tensor_scalar(
                                out=xd[:, s0:s0 + sl], in0=xraw[:, s0:s0 + sl],
                                scalar1=xs_sb[:, col + si:col + si + 1], scalar2=None,
                                op0=mybir.AluOpType.mult,
                            )
                        xt[t][e] = xd

                # ---------------- K^T and V-hat ----------------
                kps = [psA.tile([128, S], F32, tag="psA", name="psA_t") for _ in range(2)]
                vps = [psA.tile([128, KV], F32, tag="psA", name="psA_v") for _ in range(4)]
                for e in range(NE):
                    xke = xt[1][e]
                    xve = xt[2][e]
                    st = e == 0
                    sp = e == NE - 1
                    for m in range(2):
                        nc.tensor.matmul(
                            kps[m][:],
                            wk_ap(e, m * 128, (m + 1) * 128),
                            xke[:],
                            start=st,
                            stop=sp,
                        )
                    for si, (s0, sl) in enumerate(ST):
                        nc.tensor.matmul(
                            vps[si][0:sl, :],
                            xve[:, s0:s0 + sl],
                            wv_ap(e),
                            start=st,
                            stop=sp,
                        )
                # evac K^T into per-group duplicated tiles (group at rows 0-63 AND 64-127)
                kd_sb = [kdp.tile([128, S], F16, tag="kd", name="kd_t") for _ in range(G)]
                for g in range(G):
                    src = kps[g // 2][(g % 2) * 64:(g % 2) * 64 + 64, :]
                    nc.scalar.copy(kd_sb[g][0:64, :], src)
                    nc.scalar.copy(kd_sb[g][64:128, :], src)
                # evac V into [128, G, 65] tiles with ones column
                vh_sb = []
                for si, (s0, sl) in enumerate(ST):
                    t = vhp.tile([128, G, 65], F16, tag="vh", name="vh_t")
                    for g in range(G):
                        nc.vector.memset(t[:, g, 64:65], 1.0)
                    nc.scalar.copy(
                        t[0:sl, :, 0:64],
                        vps[si][0:sl, :].rearrange("p (g d) -> p g d", g=G),
                    )
                    vh_sb.append(t)

                # ---------------- Q^T (2 rounds of 4 h-tiles) ----------------
                qt_sb = [qtp.tile([128, S], F16, tag="qt", name="qt_t") for _ in range(NE)]
                for rnd in range(2):
                    qps = [psA.tile([128, S], F32, tag="psA", name="psA_t") for _ in range(4)]
                    for e in range(NE):
                        for hi in range(4):
                            ht = rnd * 4 + hi
                            nc.tensor.matmul(
                                qps[hi][:],
                                wq_ap(e, ht * 128, (ht + 1) * 128),
                                xt[0][e][:],
                                start=(e == 0),
                                stop=(e == NE - 1),
                            )
                    for hi in range(4):
                        nc.vector.tensor_copy(qt_sb[rnd * 4 + hi][:], qps[hi][:])

                # ---------------- attention per head ----------------
                ot_sb = [otp.tile([128, S], F16, tag="ot", name="ot_t") for _ in range(NE)]
                for hh in range(NH):
                    g = hh // HKV
                    base = (hh % 2) * 64
                    q_ap = qt_sb[hh // 2][base:base + 64, :]
                    p_bf = pbp.tile([128, 4, S], F16, tag="pb", name="pb_t")
                    for si, (s0, sl) in enumerate(ST):
                        sps = psA.tile([128, S], F32, tag="psA", name="psA_t")
                        nc.tensor.matmul(
                            sps[0:sl, :],
                            kd_sb[g][base:base + 64, s0:s0 + sl],
                            q_ap,
                            start=True,
                            stop=True,
                        )
                        # logits = s * 0.125 + bias (f32), then exp -> fp16 on ACT
                        p_f = pfp.tile([128, S], F32, tag="pf", name="pf_t")
                        nc.vector.scalar_tensor_tensor(
                            p_f[0:sl, :],
                            sps[0:sl, :],
                            0.125,
                            bd_sb[0:sl, hh * DW + (MD - 1 - s0):hh * DW + (MD - 1 - s0) + S],
                            op0=mybir.AluOpType.mult,
                            op1=mybir.AluOpType.add,
                        )
                        nc.scalar.activation(
                            p_bf[0:sl, si, :],
                            p_f[0:sl, :],
                            mybir.ActivationFunctionType.Exp,
                        )
                    ops = psB.tile([128, 512], F32, tag="psB", name="psB_t")
                    for si, (s0, sl) in enumerate(ST):
                        nc.tensor.matmul(
                            ops[0:65, 0:S],
                            vh_sb[si][0:sl, g, :],
                            p_bf[0:sl, si, :],
                            start=(si == 0),
                            stop=(si == 3),
                        )
                    linv = lvp.tile([1, S], F32, tag="lv", name="lv_t")
                    nc.vector.reciprocal(linv[:], ops[64:65, 0:S])
                    lbc = lbp.tile([64, S], F32, tag="lb", name="lb_t")
                    nc.gpsimd.partition_broadcast(lbc[:], linv[:])
                    nc.vector.tensor_mul(
                        ot_sb[hh // 2][base:base + 64, :],
                        ops[0:64, 0:S],
                        lbc[:],
                    )

                # ---------------- output projection + uint8 quantization ----------------
                for si, (s0, sl) in enumerate(ST):
                    accs = []
                    for n in range(2):
                        acc = psA.tile([128, 512], F32, tag="psA", name="psA_q")
                        for dt in range(NE):
                            nc.tensor.matmul(
                                acc[0:sl, :],
                                ot_sb[dt][:, s0:s0 + sl],
                                wo_ap(dt, n * 512, (n + 1) * 512),
                                start=(dt == 0),
                                stop=(dt == NE - 1),
                            )
                        accs.append(acc)
                    # per-row absmax over both halves -> scale = absmax/127
                    rm = rmp.tile([128, 4], F32, tag="rm", name="rm_t")
                    for n in range(2):
                        nc.vector.tensor_reduce(
                            rm[0:sl, n:n + 1],
                            accs[n][0:sl, :],
                            axis=mybir.AxisListType.X,
                            op=mybir.AluOpType.max,
                            apply_absolute_value=True,
                        )
                    nc.vector.tensor_tensor(
                        rm[0:sl, 2:3], rm[0:sl, 0:1], rm[0:sl, 1:2],
                        op=mybir.AluOpType.max,
                    )
                    rsc = rsp.tile([128, 1], F32, tag="rs", name="rs_t")
                    nc.vector.tensor_scalar(
                        out=rsc[0:sl, :], in0=rm[0:sl, 2:3],
                        scalar1=1e-20, scalar2=1.0 / 127.0,
                        op0=mybir.AluOpType.max, op1=mybir.AluOpType.mult,
                    )
                    nc.sync.dma_start(
                        out=bass.AP(ob, NXO + (b * S + s0) * 4, [[4, sl], [1, 4]]),
                        in_=rsc[0:sl, :].bitcast(U8),
                    )
                    nc.vector.reciprocal(rm[0:sl, 3:4], rsc[0:sl, :])
                    for n in range(2):
                        stg = osp.tile([128, 512], F16, tag="os", name="os_t")
                        nc.vector.tensor_scalar(
                            out=stg[0:sl, :], in0=accs[n][0:sl, :],
                            scalar1=rm[0:sl, 3:4], scalar2=128.0,
                            op0=mybir.AluOpType.mult, op1=mybir.AluOpType.add,
                        )
                        stu = oup.tile([128, 512], U8, tag="ou", name="ou_t")
                        nc.gpsimd.tensor_copy(stu[0:sl, :], stg[0:sl, :])
                        nc.sync.dma_start(
                            out=bass.AP(
                                ob, (b * S + s0) * E + n * 512, [[E, sl], [1, 512]]
                            ),
                            in_=stu[0:sl, :],
                        )

    nc.compile()
    return nc


_NC = None


def _get_nc():
    global _NC
    if _NC is None:
        _NC = build_nc()
    return _NC


def _host_prep(query, key, value, Wq, Wk, Wv, Wo, rel_table):
    # int8 per-(batch, tensor, channel, s-tile) symmetric quantization of x^T
    X = np.empty((B, 3, E, S), dtype=np.int8)
    SC = np.empty((B, 3, E, 4), dtype=np.float32)
    for t, a in enumerate((query, key, value)):
        at = a.transpose(0, 2, 1)  # [B, E, S] view
        for si, (s0, sl) in enumerate(ST):
            blk = at[:, :, s0:s0 + sl]
            amax = np.maximum(np.abs(blk).max(axis=2), 1e-2)  # [B, E]; floor keeps fp16 scale normal
            # round the scale to fp16 BEFORE quantizing so device dequant
            # (which reads the fp16 scale) is exactly consistent
            sc = (amax / 127.0).astype(np.float16).astype(np.float32)
            SC[:, t, :, si] = sc
            q = np.rint(blk / sc[:, :, None])
            np.clip(q, -127, 127, out=q)
            X[:, t, :, s0:s0 + sl] = q
    # xsc layout: [p, ((b*3+t)*NE+e)*4+si] per core (b local)
    xs_all = np.ascontiguousarray(
        SC.reshape(B, 3, NE, 128, 4).transpose(3, 0, 1, 2, 4).reshape(128, B * 3 * NE * 4)
    ).astype(np.float16)

    wbm = np.empty((E, WB_W), dtype=np.float16)
    wbm[:, WQ0:WQ0 + H] = Wq
    wbm[:, WK0:WK0 + KV] = Wk
    wbm[:, WV0:WV0 + KV] = Wv
    wbm[:, WO0:WO0 + E] = Wo
    # pd[h, m] = rel_table[1093 - m, h] for m in [127, 1093], else 0
    pdv = np.zeros((NH, PW), dtype=np.float16)
    pdv[:, 127:127 + TW] = rel_table[::-1, :].T

    ncols = B_LOC * 3 * NE * 4
    in_maps = []
    for c in range(N_CORES):
        sl = slice(c * B_LOC, (c + 1) * B_LOC)
        wp_c = np.concatenate(
            [wbm[c * 128:(c + 1) * 128].reshape(-1), pdv.reshape(-1)]
        )
        in_maps.append(
            {
                "xb": X[sl],
                "xsc": np.ascontiguousarray(xs_all[:, c * ncols:(c + 1) * ncols]),
                "wp": wp_c,
            }
        )
    return in_maps


def _run(inputs, trace=False):
    nc = _get_nc()
    in_maps = _host_prep(**inputs)
    res = run_bass_kernel_spmd(
        nc, in_maps, list(range(N_CORES)), trace=trace
    )
    outs = []
    for r in res.results:
        blob = r["ob"]
        u = blob[:NXO].reshape(B_LOC, S, E).astype(np.float32)
        sc = blob[NXO:].view(np.float32).reshape(B_LOC, S)
        u -= 128.0
        u *= sc[:, :, None]
        outs.append(u)
    outp = np.concatenate(outs, axis=0)
    return outp, res


def kernel(query, key, value, Wq, Wk, Wv, Wo, rel_table):
    outp, _ = _run(
        dict(
            query=np.asarray(query),
            key=np.asarray(key),
            value=np.asarray(value),
            Wq=np.asarray(Wq),
            Wk=np.asarray(Wk),
            Wv=np.asarray(Wv),
            Wo=np.asarray(Wo),
            rel_table=np.asarray(rel_table),
        )
    )
    return outp
